# revision 13
# baseline (speedup 1.0000x reference)
"""Causal multi-head attention (B=4, T=2048, C=384, 6 heads of 64) on 8 trn2 cores.

Sharding: 24 (batch, head) pairs -> 8 cores; core c handles batch c//2 and
heads [3*(c%2), 3*(c%2)+3). Each core computes q/k/v projections for its 3
heads, causal softmax(q k^T / 8) v, and a PARTIAL output projection
ctx_heads @ Wo_heads. Host sums the two partials per batch and adds the
exactly-folded bias terms (bv @ Wo + bo; softmax weights sum to 1 so a v-bias
contributes bv @ Wo to every row).

Requires bq == bk == 0 (true for this problem: spec fill=zeros).

v2 structural changes vs baseline:
 - softmax normalization: sums gathered via direct SBUF->SBUF DMA,
   reciprocal_approx_fast (5x faster than iterative divide), and the
   64-partition broadcast done with a tiny PE matmul (one-hot selector)
   instead of DRAM round-trip DMAs.
 - PSUM->SBUF evacuations split between ScalarE and VectorE.
 - input DMAs batched (3 for x^T, 1 per weight tensor); vones -> memset.
 - one output DMA per t-chunk (4 total) instead of 16.
 - dummy exp at start so the ACT table load happens during the DMA phase.
"""

import math
from contextlib import ExitStack

import ml_dtypes
import numpy as np

BF16NP = ml_dtypes.bfloat16

B, T, C = 4, 2048, 384
NH, D = 6, 64          # total heads, head dim
HPC = 3                # heads per core
NCORES = 8
NKC = C // 128         # 3 contraction chunks for the projections
NTB = T // 128         # 16 row blocks
TCW = 512              # t-chunk width for the attention loop
NTC = T // TCW         # 4 t-chunks

_CACHED_NC = None


def build_nc():
    import concourse.bass as bass
    import concourse.mybir as mybir
    import concourse.tile as tile
    from concourse import bacc

    F32 = mybir.dt.float32
    F32R = mybir.dt.float32r
    FP8 = mybir.dt.float8e4
    I32 = mybir.dt.int32
    DR = mybir.MatmulPerfMode.DoubleRow
    EXPB = -1.5  # global shift: keeps exp(s) < 448 for fp8; cancels in softmax
    BF16 = mybir.dt.bfloat16
    EXPF = mybir.ActivationFunctionType.Exp
    COPYF = mybir.ActivationFunctionType.Copy

    nc = bacc.Bacc("TRN2", target_bir_lowering=False, debug=False)

    xt = nc.dram_tensor("xt", [C, T], BF16, kind="ExternalInput")
    wqk = nc.dram_tensor("wqk", [C, 512], BF16, kind="ExternalInput")
    wv = nc.dram_tensor("wv", [C, 192], BF16, kind="ExternalInput")
    wo = nc.dram_tensor("wo", [HPC * D, 384], BF16, kind="ExternalInput")
    zt = nc.dram_tensor("zt", [128, 512], BF16, kind="ExternalInput")
    sel = nc.dram_tensor("sel", [HPC, HPC * D], BF16, kind="ExternalInput")
    out = nc.dram_tensor("out", [T, C], BF16, kind="ExternalOutput")

    with ExitStack() as ctx:
        tc = ctx.enter_context(tile.TileContext(nc))
        const = ctx.enter_context(tc.tile_pool(name="const", bufs=1))
        xpool = ctx.enter_context(tc.tile_pool(name="xp", bufs=1))
        qkpool = ctx.enter_context(tc.tile_pool(name="qkp", bufs=1))
        vpool = ctx.enter_context(tc.tile_pool(name="vp", bufs=1))
        expp = ctx.enter_context(tc.tile_pool(name="expp", bufs=6))
        cxp = ctx.enter_context(tc.tile_pool(name="cxp", bufs=1))
        rpool = ctx.enter_context(tc.tile_pool(name="rp", bufs=2))
        opool = ctx.enter_context(tc.tile_pool(name="op", bufs=2))
        ps_s = ctx.enter_context(tc.tile_pool(name="ps_s", bufs=2, space="PSUM"))
        ps_c = ctx.enter_context(tc.tile_pool(name="ps_c", bufs=3, space="PSUM"))
        ps_o = ctx.enter_context(tc.tile_pool(name="ps_o", bufs=1, space="PSUM"))

        # ---- constants (batched DMAs) ----
        wqk_all = const.tile([128, NKC * 512], BF16, tag="wqk")
        nc.sync.dma_start(
            out=wqk_all.rearrange("p (kc c) -> p kc c", kc=NKC),
            in_=wqk.rearrange("(kc p) c -> p kc c", p=128),
        )
        wqk_sb = [wqk_all[:, kc * 512:(kc + 1) * 512] for kc in range(NKC)]

        zt_sb = const.tile([128, 512], BF16, tag="zt")
        nc.sync.dma_start(out=zt_sb, in_=zt[:, :])
        zt8 = const.tile([128, 512], FP8, tag="zt8")
        nc.vector.tensor_copy(out=zt8[:, :], in_=zt_sb[:, :])
        sel_sb = const.tile([HPC, HPC * D], BF16, tag="sel")
        nc.sync.dma_start(out=sel_sb, in_=sel[:, :])

        expb = const.tile([128, 1], F32, tag="expb")
        nc.vector.memset(expb[:, :], EXPB)

        # dummy exp: forces the ACT table load during the DMA phase
        dumm = const.tile([1, 8], BF16, tag="dumm")
        nc.scalar.activation(dumm[:, :], zt_sb[0:1, 0:8], EXPF)

        # ---- x^T in SBUF: 3 [128, 2048] row chunks (1 DMA each) ----
        xts = []
        for kc in range(NKC):
            t_ = xpool.tile([128, T], BF16, tag=f"xt{kc}")
            nc.sync.dma_start(out=t_, in_=xt[kc * 128:(kc + 1) * 128, :])
            xts.append(t_)

        wv_all = const.tile([128, NKC * 192], BF16, tag="wv")
        nc.sync.dma_start(
            out=wv_all.rearrange("p (kc c) -> p kc c", kc=NKC),
            in_=wv.rearrange("(kc p) c -> p kc c", p=128),
        )
        wv_sb = [wv_all[:, kc * 192:(kc + 1) * 192] for kc in range(NKC)]

        wo_all = const.tile([64, HPC * 384], BF16, tag="wo")
        nc.sync.dma_start(
            out=wo_all.rearrange("p (h c) -> p h c", h=HPC),
            in_=wo.rearrange("(h p) c -> p h c", p=64),
        )
        wo_sb = [wo_all[:, h * 384:(h + 1) * 384] for h in range(HPC)]

        # ---- projections: qT/kT packed [d(2 heads), T] ----
        # wqk columns: mt0=[q0|q1] mt1=[k0|k1] mt2=[q2|q2] mt3=[k2|k2]
        # (q columns pre-scaled by 1/sqrt(D) on host)
        qk_names = ["qT01", "kT01", "qT22", "kT22"]
        qkT = {}
        for mt, name in enumerate(qk_names):
            qkT[name] = qkpool.tile([128, T], BF16, tag=name, name=name)
        for mt, name in enumerate(qk_names):
            for nch in range(4):
                ps = ps_s.tile([128, 1024], F32, tag="S")
                for kc in range(NKC):
                    nc.tensor.matmul(
                        ps[:, 0:512],
                        lhsT=wqk_sb[kc][:, mt * 128:(mt + 1) * 128],
                        rhs=xts[kc][:, nch * 512:(nch + 1) * 512],
                        start=(kc == 0),
                        stop=(kc == NKC - 1),
                    )
                dst = qkT[name][:, nch * 512:(nch + 1) * 512]
                if (mt * 4 + nch) % 2 == 0:
                    nc.vector.tensor_copy(out=dst, in_=ps[:, 0:512])
                else:
                    nc.scalar.activation(dst, ps[:, 0:512], COPYF)

        # ---- v in natural [s, d'] layout + ones column per head (66 wide) ----
        v_sb = vpool.tile([128, NTB, 208], FP8, tag="vsb")
        ones_view = v_sb[:, :, 0:198].rearrange("p s (h e) -> p s h e", e=66)[:, :, :, 64:66]
        nc.vector.memset(ones_view, 1.0)
        v_bf = vpool.tile([128, 4, 3 * 66], BF16, tag="vbf")
        ones_bf = v_bf.rearrange("p s (h e) -> p s h e", e=66)[:, :, :, 64:66]
        nc.vector.memset(ones_bf, 1.0)
        for tb in range(NTB):
            ps = ps_s.tile([128, 1024], F32, tag="S")
            for kc in range(NKC):
                nc.tensor.matmul(
                    ps[:, 0:192],
                    lhsT=xts[kc][:, tb * 128:(tb + 1) * 128],
                    rhs=wv_sb[kc][:, :],
                    start=(kc == 0),
                    stop=(kc == NKC - 1),
                )
            dst = v_sb[:, tb, 0:198].rearrange("p (h e) -> p h e", e=66)[:, :, 0:64]
            src = ps[:, 0:192].rearrange("p (h e) -> p h e", e=64)
            if tb % 2 == 0:
                nc.vector.tensor_copy(out=dst, in_=src)
            else:
                nc.scalar.activation(dst, src, COPYF)
            if tb < 4:
                dbf = v_bf[:, tb, :].rearrange("p (h e) -> p h e", e=66)[:, :, 0:64]
                if tb % 2 == 0:
                    nc.scalar.activation(dbf, src, COPYF)
                else:
                    nc.vector.tensor_copy(out=dbf, in_=src)

        # ---- per-head normalized ctx^T [64, T] ----
        ctxT = [
            cxp.tile([64, T], BF16, tag=f"ctxT{h}", name=f"ctxT{h}")
            for h in range(HPC)
        ]

        def mask_exp_block(e_tile, col0, sbm):
            """Causal-mask the exp'd S^T block at e_tile[:, col0:col0+512].

            sbm = s_block_start - t_chunk_start (>= 0 on diagonal blocks).
            Cols [col0, col0+sbm) are fully above the diagonal (-> *0) and
            cols [col0+sbm, col0+sbm+128) are triangular; zt_sb is laid out
            as [384 zeros | 128-wide triangle] so one suffix slice covers
            both regions in a single multiply."""
            w = sbm + 128
            nc.vector.tensor_mul(
                e_tile[:, col0:col0 + w],
                e_tile[:, col0:col0 + w],
                zt_sb[:, 512 - w:512],
            )

        for tci in range(NTC):
            tsl = slice(tci * TCW, (tci + 1) * TCW)
            nsb = 4 * tci + 4
            cps = [
                ps_c.tile([128, TCW], F32, tag="ctx", name=f"cps{tci}_{h}")
                for h in range(HPC)
            ]
            is_bf = tci == 0  # first t-chunk: few attended positions; fp8
            # V-noise does not average out -> keep it in bf16
            for pr in range(nsb // 2):
                sb0 = 2 * pr
                # epair layout: [p, head, j(sb parity), t]
                if is_bf:
                    ep = expp.tile([128, HPC, 2, TCW], BF16, tag="E0", bufs=2)
                    ztt = zt_sb
                else:
                    ep = expp.tile([128, HPC, 2, TCW], FP8, tag="E")
                    ztt = zt8
                # --- heads 0,1: per sb, row-tiled score pair (K=64 each) ---
                for j in range(2):
                    sbj = sb0 + j
                    sbm = sbj * 128 - tci * TCW  # >= 0 on diagonal blocks
                    s01 = ps_s.tile([128, 1024], F32, tag="S")
                    for hh in range(2):
                        psl = slice(hh * 64, (hh + 1) * 64)
                        nc.tensor.matmul(
                            s01[:, hh * 512:(hh + 1) * 512],
                            lhsT=qkT["kT01"][psl, sbj * 128:(sbj + 1) * 128],
                            rhs=qkT["qT01"][psl, tsl],
                            start=True,
                            stop=True,
                        )
                    # skip exp of fully-masked cols [0, sbm) on diagonal blocks
                    # (the mask multiply zeroes them regardless)
                    lo = max(sbm, 0)
                    nc.scalar.activation(
                        ep[:, 0:2, j, lo:TCW],
                        s01.rearrange("p (h t) -> p h t", h=2)[:, :, lo:TCW],
                        EXPF,
                        bias=expb[:, 0:1],
                    )
                    if sbm >= 0:
                        for hh in range(2):
                            if sbm > 0:
                                nc.vector.memset(
                                    ep[:, hh, j, 0:sbm].bitcast(I32), 0
                                )
                            nc.vector.tensor_mul(
                                ep[:, hh, j, sbm:sbm + 128],
                                ep[:, hh, j, sbm:sbm + 128],
                                ztt[:, 384:512],
                            )
                # --- head 2: both sbs of the pair, row-tiled (dup packing) ---
                s2 = ps_s.tile([128, 1024], F32, tag="S")
                for j in range(2):
                    sbj = sb0 + j
                    psl = slice(j * 64, (j + 1) * 64)
                    nc.tensor.matmul(
                        s2[:, j * 512:(j + 1) * 512],
                        lhsT=qkT["kT22"][psl, sbj * 128:(sbj + 1) * 128],
                        rhs=qkT["qT22"][psl, tsl],
                        start=True,
                        stop=True,
                    )
                sbm0 = sb0 * 128 - tci * TCW
                if sbm0 >= 256:
                    for j in range(2):
                        lo = sbm0 + j * 128
                        nc.scalar.activation(
                            ep[:, 2, j, lo:TCW],
                            s2[:, j * 512 + lo:(j + 1) * 512],
                            EXPF,
                            bias=expb[:, 0:1],
                        )
                else:
                    nc.scalar.activation(
                        ep[:, 2, :, :],
                        s2.rearrange("p (j t) -> p j t", j=2),
                        EXPF,
                        bias=expb[:, 0:1],
                    )
                for j in range(2):
                    sbmj = (sb0 + j) * 128 - tci * TCW
                    if sbmj >= 0:
                        if sbmj > 0:
                            nc.vector.memset(ep[:, 2, j, 0:sbmj].bitcast(I32), 0)
                        nc.vector.tensor_mul(
                            ep[:, 2, j, sbmj:sbmj + 128],
                            ep[:, 2, j, sbmj:sbmj + 128],
                            ztt[:, 384:512],
                        )
                # --- PV: one DoubleRow matmul per head covers both sbs
                #     (bf16 path: two normal matmuls) ---
                for h in range(HPC):
                    if is_bf:
                        for j in range(2):
                            nc.tensor.matmul(
                                cps[h][0:66, :],
                                lhsT=v_bf[:, sb0 + j, h * 66:(h + 1) * 66],
                                rhs=ep[:, h, j, :],
                                start=(sb0 + j == 0),
                                stop=(sb0 + j == nsb - 1),
                            )
                    else:
                        nc.tensor.matmul(
                            cps[h][0:66, :],
                            lhsT=v_sb[:, sb0:sb0 + 2, h * 66:(h + 1) * 66],
                            rhs=ep[:, h, :, :],
                            start=(pr == 0),
                            stop=(pr == nsb // 2 - 1),
                            perf_mode=DR,
                        )
            # --- normalize: evac unnormalized ctx+sums; fast reciprocal;
            #     broadcast 1/sums to 64 partitions via a one-hot PE matmul ---
            cues = []
            if tci == NTC - 1:
                # keep the PE HAM-warm through the normalize chain so the
                # final out-projection runs at 2.4 GHz
                dum_ps = ps_s.tile([128, 1024], F32, tag="S", name="dumps")
                for dk in range(8):
                    nc.tensor.matmul(
                        dum_ps[:, 0:512],
                        lhsT=zt_sb[:, 0:128],
                        rhs=zt_sb[:, 0:512],
                        start=(dk == 0),
                        stop=(dk == 7),
                    )
            sall = rpool.tile([HPC, TCW], F32, tag="sall", name=f"sall{tci}")
            for h in range(HPC):
                cue = rpool.tile([66, TCW], F32, tag=f"cue{h}", name=f"cue{tci}_{h}")
                if h == 1 and tci == NTC - 1:  # scalar is idle on the tail
                    nc.scalar.activation(cue[:, :], cps[h][0:66, :], COPYF)
                else:
                    nc.vector.tensor_copy(out=cue[:, :], in_=cps[h][0:66, :])
                nc.sync.dma_start(out=sall[h:h + 1, :], in_=cue[64:65, :])
                cues.append(cue)
            rall = rpool.tile([HPC, TCW], F32, tag="rall", name=f"rall{tci}")
            nc.vector.reciprocal_approx_fast(out=rall[:, :], in_=sall[:, :])
            rall_bf = rpool.tile([HPC, TCW], BF16, tag="rallbf", name=f"rbf{tci}")
            nc.vector.tensor_copy(out=rall_bf[:, :], in_=rall[:, :])
            # broadcast h0 -> rows 0:64 and h1 -> rows 64:128 of one bank
            # (concurrent col-tiled matmuls), then h2 on the next rotation.
            rec01 = ps_o.tile([128, TCW], F32, tag="O", name=f"rec01_{tci}")
            for hh in range(2):
                nc.tensor.matmul(
                    rec01[hh * 64:(hh + 1) * 64, :],
                    lhsT=sel_sb[:, hh * 64:(hh + 1) * 64],
                    rhs=rall_bf[:, :],
                    start=True,
                    stop=True,
                )
            nc.vector.tensor_mul(ctxT[0][:, tsl], cues[0][0:64, :], rec01[0:64, :])
            nc.vector.tensor_mul(ctxT[1][:, tsl], cues[1][0:64, :], rec01[64:128, :])
            psx = ps_c if tci == NTC - 1 else ps_o
            tgx = "ctx" if tci == NTC - 1 else "O"
            rec2 = psx.tile([128, TCW], F32, tag=tgx, name=f"rec2_{tci}")
            nc.tensor.matmul(
                rec2[0:64, :],
                lhsT=sel_sb[:, 128:192],
                rhs=rall_bf[:, :],
                start=True,
                stop=True,
            )
            nc.vector.tensor_mul(ctxT[2][:, tsl], cues[2][0:64, :], rec2[0:64, :])
            # --- output projection for this t-chunk; single DMA out ---
            osb = opool.tile([128, 4 * 384], BF16, tag="osb", name=f"osb{tci}")
            for j, tb in enumerate(range(4 * tci, 4 * tci + 4)):
                po = psx.tile([128, 512], F32, tag=tgx, name=f"po{tb}")
                for h in range(HPC):
                    nc.tensor.matmul(
                        po[:, 0:384],
                        lhsT=ctxT[h][:, tb * 128:(tb + 1) * 128],
                        rhs=wo_sb[h][:, :],
                        start=(h == 0),
                        stop=(h == HPC - 1),
                    )
                dst = osb[:, j * 384:(j + 1) * 384]
                if tci == NTC - 1 and j % 2 == 1:
                    nc.scalar.activation(dst, po[:, 0:384], COPYF)
                else:
                    nc.vector.tensor_copy(out=dst, in_=po[:, 0:384])
                if tci == NTC - 1:
                    nc.sync.dma_start(
                        out=out[tb * 128:(tb + 1) * 128, :],
                        in_=osb[:, j * 384:(j + 1) * 384],
                    )
                elif j == 1 or j == 3:
                    jh = j - 1
                    nc.sync.dma_start(
                        out=out[
                            tci * 512 + jh * 128:tci * 512 + (jh + 2) * 128, :
                        ].rearrange("(jj p) c -> p jj c", p=128),
                        in_=osb.rearrange("p (jj c) -> p jj c", jj=4)[
                            :, jh:jh + 2, :
                        ],
                    )

    return nc


def get_nc():
    global _CACHED_NC
    if _CACHED_NC is None:
        nc = build_nc()
        nc.finalize()
        _CACHED_NC = nc
    return _CACHED_NC


def make_core_inputs(x, Wq, bq, Wk, bk, Wv, bv, Wo, bo):
    """Host-side shard prep. Returns (in_maps, host_add) where host_add[384]
    is added to every output row (exact fold of bv/bo)."""
    scale = 1.0 / math.sqrt(D)
    assert np.all(bq == 0.0) and np.all(bk == 0.0), "kernel assumes bq=bk=0"
    host_add = (bv.astype(np.float64) @ Wo.astype(np.float64) + bo).astype(np.float32)

    si = np.arange(128)[:, None]
    tj = np.arange(128)[None, :]
    zt = np.zeros((128, 512), dtype=np.float32)
    zt[:, 384:512] = (si <= tj).astype(np.float32)

    sel = np.zeros((HPC, HPC * D), dtype=np.float32)
    for h in range(HPC):
        sel[h, h * D:(h + 1) * D] = 1.0

    in_maps = []
    for core in range(NCORES):
        b = core // 2
        h0 = HPC * (core % 2)  # first head (0 or 3)
        cs = slice(h0 * D, (h0 + HPC) * D)
        wq_s = (Wq[:, cs] * scale).astype(np.float32)
        wk_s = Wk[:, cs].astype(np.float32)
        wqk = np.concatenate(
            [
                wq_s[:, 0:128],
                wk_s[:, 0:128],
                np.tile(wq_s[:, 128:192], (1, 2)),
                np.tile(wk_s[:, 128:192], (1, 2)),
            ],
            axis=1,
        )
        in_maps.append(
            {
                "xt": np.ascontiguousarray(x[b].T).astype(BF16NP),
                "wqk": np.ascontiguousarray(wqk).astype(BF16NP),
                "wv": np.ascontiguousarray(Wv[:, cs]).astype(BF16NP),
                "wo": np.ascontiguousarray(Wo[cs, :]).astype(BF16NP),
                "zt": zt.astype(BF16NP),
                "sel": sel.astype(BF16NP),
            }
        )
    return in_maps, host_add


def kernel(x, Wq, bq, Wk, bk, Wv, bv, Wo, bo, _trace=False):
    x = np.asarray(x, dtype=np.float32)
    Wq, bq = np.asarray(Wq, np.float32), np.asarray(bq, np.float32)
    Wk, bk = np.asarray(Wk, np.float32), np.asarray(bk, np.float32)
    Wv, bv = np.asarray(Wv, np.float32), np.asarray(bv, np.float32)
    Wo, bo = np.asarray(Wo, np.float32), np.asarray(bo, np.float32)

    from concourse.bass_utils import run_bass_kernel_spmd

    nc = get_nc()
    in_maps, host_add = make_core_inputs(x, Wq, bq, Wk, bk, Wv, bv, Wo, bo)
    res = run_bass_kernel_spmd(
        nc, in_maps, core_ids=list(range(NCORES)), trace=_trace
    )
    out = np.empty((B, T, C), dtype=np.float32)
    for b in range(B):
        out[b] = (
            res.results[2 * b]["out"].astype(np.float32)
            + res.results[2 * b + 1]["out"].astype(np.float32)
            + host_add
        )
    if _trace:
        return out, res
    return out


# revision 14
# speedup vs baseline: 1.2439x; 1.2439x over previous
"""Causal multi-head attention (B=4, T=2048, C=384, 6 heads of 64) on 8 trn2 cores.

Sharding: 24 (batch, head) pairs -> 8 cores; core c handles batch c//2 and
heads [3*(c%2), 3*(c%2)+3). Each core computes q/k/v projections for its 3
heads, causal softmax(q k^T / 8) v, and a PARTIAL output projection
ctx_heads @ Wo_heads. Host sums the two partials per batch and adds the
exactly-folded bias terms (bv @ Wo + bo; softmax weights sum to 1 so a v-bias
contributes bv @ Wo to every row).

Requires bq == bk == 0 (true for this problem: spec fill=zeros).

v2 structural changes vs baseline:
 - softmax normalization: sums gathered via direct SBUF->SBUF DMA,
   reciprocal_approx_fast (5x faster than iterative divide), and the
   64-partition broadcast done with a tiny PE matmul (one-hot selector)
   instead of DRAM round-trip DMAs.
 - PSUM->SBUF evacuations split between ScalarE and VectorE.
 - input DMAs batched (3 for x^T, 1 per weight tensor); vones -> memset.
 - one output DMA per t-chunk (4 total) instead of 16.
 - dummy exp at start so the ACT table load happens during the DMA phase.
"""

import math
from contextlib import ExitStack

import ml_dtypes
import numpy as np

BF16NP = ml_dtypes.bfloat16

B, T, C = 4, 2048, 384
NH, D = 6, 64          # total heads, head dim
HPC = 3                # heads per core
NCORES = 8
NKC = C // 128         # 3 contraction chunks for the projections
NTB = T // 128         # 16 row blocks
TCW = 512              # t-chunk width for the attention loop
NTC = T // TCW         # 4 t-chunks

_CACHED_NC = None


def build_nc():
    import concourse.bass as bass
    import concourse.mybir as mybir
    import concourse.tile as tile
    from concourse import bacc

    F32 = mybir.dt.float32
    F32R = mybir.dt.float32r
    FP8 = mybir.dt.float8e4
    I32 = mybir.dt.int32
    DR = mybir.MatmulPerfMode.DoubleRow
    EXPB = -1.5  # global shift: keeps exp(s) < 448 for fp8; cancels in softmax
    BF16 = mybir.dt.bfloat16
    EXPF = mybir.ActivationFunctionType.Exp
    COPYF = mybir.ActivationFunctionType.Copy

    nc = bacc.Bacc("TRN2", target_bir_lowering=False, debug=False)

    xt = nc.dram_tensor("xt", [C, T], BF16, kind="ExternalInput")
    wqk = nc.dram_tensor("wqk", [C, 512], BF16, kind="ExternalInput")
    wv = nc.dram_tensor("wv", [C, 192], BF16, kind="ExternalInput")
    wo = nc.dram_tensor("wo", [HPC * D, 384], BF16, kind="ExternalInput")
    zt = nc.dram_tensor("zt", [128, 512], BF16, kind="ExternalInput")
    sel = nc.dram_tensor("sel", [HPC, HPC * D], BF16, kind="ExternalInput")
    out = nc.dram_tensor("out", [T, C], F32, kind="ExternalOutput")

    with ExitStack() as ctx:
        tc = ctx.enter_context(tile.TileContext(nc))
        const = ctx.enter_context(tc.tile_pool(name="const", bufs=1))
        xpool = ctx.enter_context(tc.tile_pool(name="xp", bufs=1))
        qkpool = ctx.enter_context(tc.tile_pool(name="qkp", bufs=1))
        vpool = ctx.enter_context(tc.tile_pool(name="vp", bufs=1))
        expp = ctx.enter_context(tc.tile_pool(name="expp", bufs=6))
        cxp = ctx.enter_context(tc.tile_pool(name="cxp", bufs=1))
        rpool = ctx.enter_context(tc.tile_pool(name="rp", bufs=2))
        opool = ctx.enter_context(tc.tile_pool(name="op", bufs=2))
        ps_s = ctx.enter_context(tc.tile_pool(name="ps_s", bufs=2, space="PSUM"))
        ps_c = ctx.enter_context(tc.tile_pool(name="ps_c", bufs=3, space="PSUM"))
        ps_o = ctx.enter_context(tc.tile_pool(name="ps_o", bufs=1, space="PSUM"))

        # ---- constants (batched DMAs) ----
        wqk_all = const.tile([128, NKC * 512], BF16, tag="wqk")
        nc.sync.dma_start(
            out=wqk_all.rearrange("p (kc c) -> p kc c", kc=NKC),
            in_=wqk.rearrange("(kc p) c -> p kc c", p=128),
        )
        wqk_sb = [wqk_all[:, kc * 512:(kc + 1) * 512] for kc in range(NKC)]

        zt_sb = const.tile([128, 512], BF16, tag="zt")
        nc.sync.dma_start(out=zt_sb, in_=zt[:, :])
        zt8 = const.tile([128, 512], FP8, tag="zt8")
        nc.vector.tensor_copy(out=zt8[:, :], in_=zt_sb[:, :])
        sel_sb = const.tile([HPC, HPC * D], BF16, tag="sel")
        nc.sync.dma_start(out=sel_sb, in_=sel[:, :])

        expb = const.tile([128, 1], F32, tag="expb")
        nc.vector.memset(expb[:, :], EXPB)

        # dummy exp: forces the ACT table load during the DMA phase
        dumm = const.tile([1, 8], BF16, tag="dumm")
        nc.scalar.activation(dumm[:, :], zt_sb[0:1, 0:8], EXPF)

        # ---- x^T in SBUF: 3 [128, 2048] row chunks (1 DMA each) ----
        xts = []
        for kc in range(NKC):
            t_ = xpool.tile([128, T], BF16, tag=f"xt{kc}")
            nc.sync.dma_start(out=t_, in_=xt[kc * 128:(kc + 1) * 128, :])
            xts.append(t_)

        wv_all = const.tile([128, NKC * 192], BF16, tag="wv")
        nc.sync.dma_start(
            out=wv_all.rearrange("p (kc c) -> p kc c", kc=NKC),
            in_=wv.rearrange("(kc p) c -> p kc c", p=128),
        )
        wv_sb = [wv_all[:, kc * 192:(kc + 1) * 192] for kc in range(NKC)]

        wo_all = const.tile([64, HPC * 384], BF16, tag="wo")
        nc.sync.dma_start(
            out=wo_all.rearrange("p (h c) -> p h c", h=HPC),
            in_=wo.rearrange("(h p) c -> p h c", p=64),
        )
        wo_sb = [wo_all[:, h * 384:(h + 1) * 384] for h in range(HPC)]

        # ---- projections: qT/kT packed [d(2 heads), T] ----
        # wqk columns: mt0=[q0|q1] mt1=[k0|k1] mt2=[q2|q2] mt3=[k2|k2]
        # (q columns pre-scaled by 1/sqrt(D) on host)
        qk_names = ["qT01", "kT01", "qT22", "kT22"]
        qkT = {}
        for mt, name in enumerate(qk_names):
            qkT[name] = qkpool.tile([128, T], BF16, tag=name, name=name)
        for mt, name in enumerate(qk_names):
            for nch in range(4):
                ps = ps_s.tile([128, 1024], F32, tag="S")
                for kc in range(NKC):
                    nc.tensor.matmul(
                        ps[:, 0:512],
                        lhsT=wqk_sb[kc][:, mt * 128:(mt + 1) * 128],
                        rhs=xts[kc][:, nch * 512:(nch + 1) * 512],
                        start=(kc == 0),
                        stop=(kc == NKC - 1),
                    )
                dst = qkT[name][:, nch * 512:(nch + 1) * 512]
                if (mt * 4 + nch) % 2 == 0:
                    nc.vector.tensor_copy(out=dst, in_=ps[:, 0:512])
                else:
                    nc.scalar.activation(dst, ps[:, 0:512], COPYF)

        # ---- v in natural [s, d'] layout + ones column per head (66 wide) ----
        v_sb = vpool.tile([128, NTB, 208], FP8, tag="vsb")
        ones_view = v_sb[:, :, 0:198].rearrange("p s (h e) -> p s h e", e=66)[:, :, :, 64:66]
        nc.vector.memset(ones_view, 1.0)
        v_bf = vpool.tile([128, 4, 3 * 66], BF16, tag="vbf")
        ones_bf = v_bf.rearrange("p s (h e) -> p s h e", e=66)[:, :, :, 64:66]
        nc.vector.memset(ones_bf, 1.0)
        for tb in range(NTB):
            ps = ps_s.tile([128, 1024], F32, tag="S")
            for kc in range(NKC):
                nc.tensor.matmul(
                    ps[:, 0:192],
                    lhsT=xts[kc][:, tb * 128:(tb + 1) * 128],
                    rhs=wv_sb[kc][:, :],
                    start=(kc == 0),
                    stop=(kc == NKC - 1),
                )
            dst = v_sb[:, tb, 0:198].rearrange("p (h e) -> p h e", e=66)[:, :, 0:64]
            src = ps[:, 0:192].rearrange("p (h e) -> p h e", e=64)
            if tb % 2 == 0:
                nc.vector.tensor_copy(out=dst, in_=src)
            else:
                nc.scalar.activation(dst, src, COPYF)
            if tb < 4:
                dbf = v_bf[:, tb, :].rearrange("p (h e) -> p h e", e=66)[:, :, 0:64]
                if tb % 2 == 0:
                    nc.scalar.activation(dbf, src, COPYF)
                else:
                    nc.vector.tensor_copy(out=dbf, in_=src)

        # ---- per-head normalized ctx^T [64, T] ----
        ctxT = [
            cxp.tile([64, T], BF16, tag=f"ctxT{h}", name=f"ctxT{h}")
            for h in range(HPC)
        ]

        def mask_exp_block(e_tile, col0, sbm):
            """Causal-mask the exp'd S^T block at e_tile[:, col0:col0+512].

            sbm = s_block_start - t_chunk_start (>= 0 on diagonal blocks).
            Cols [col0, col0+sbm) are fully above the diagonal (-> *0) and
            cols [col0+sbm, col0+sbm+128) are triangular; zt_sb is laid out
            as [384 zeros | 128-wide triangle] so one suffix slice covers
            both regions in a single multiply."""
            w = sbm + 128
            nc.vector.tensor_mul(
                e_tile[:, col0:col0 + w],
                e_tile[:, col0:col0 + w],
                zt_sb[:, 512 - w:512],
            )

        for tci in range(NTC):
            tsl = slice(tci * TCW, (tci + 1) * TCW)
            nsb = 4 * tci + 4
            cps = [
                ps_c.tile([128, TCW], F32, tag="ctx", name=f"cps{tci}_{h}")
                for h in range(HPC)
            ]
            is_bf = tci == 0  # first t-chunk: few attended positions; fp8
            # V-noise does not average out -> keep it in bf16
            for pr in range(nsb // 2):
                sb0 = 2 * pr
                # epair layout: [p, head, j(sb parity), t]
                if is_bf:
                    ep = expp.tile([128, HPC, 2, TCW], BF16, tag="E0", bufs=2)
                    ztt = zt_sb
                else:
                    ep = expp.tile([128, HPC, 2, TCW], FP8, tag="E")
                    ztt = zt8
                # --- heads 0,1: per sb, row-tiled score pair (K=64 each) ---
                for j in range(2):
                    sbj = sb0 + j
                    sbm = sbj * 128 - tci * TCW  # >= 0 on diagonal blocks
                    s01 = ps_s.tile([128, 1024], F32, tag="S")
                    for hh in range(2):
                        psl = slice(hh * 64, (hh + 1) * 64)
                        nc.tensor.matmul(
                            s01[:, hh * 512:(hh + 1) * 512],
                            lhsT=qkT["kT01"][psl, sbj * 128:(sbj + 1) * 128],
                            rhs=qkT["qT01"][psl, tsl],
                            start=True,
                            stop=True,
                        )
                    # skip exp of fully-masked cols [0, sbm) on diagonal blocks
                    # (the mask multiply zeroes them regardless)
                    lo = max(sbm, 0)
                    nc.scalar.activation(
                        ep[:, 0:2, j, lo:TCW],
                        s01.rearrange("p (h t) -> p h t", h=2)[:, :, lo:TCW],
                        EXPF,
                        bias=expb[:, 0:1],
                    )
                    if sbm >= 0:
                        for hh in range(2):
                            if sbm > 0:
                                nc.vector.memset(
                                    ep[:, hh, j, 0:sbm].bitcast(I32), 0
                                )
                            nc.vector.tensor_mul(
                                ep[:, hh, j, sbm:sbm + 128],
                                ep[:, hh, j, sbm:sbm + 128],
                                ztt[:, 384:512],
                            )
                # --- head 2: both sbs of the pair, row-tiled (dup packing) ---
                s2 = ps_s.tile([128, 1024], F32, tag="S")
                for j in range(2):
                    sbj = sb0 + j
                    psl = slice(j * 64, (j + 1) * 64)
                    nc.tensor.matmul(
                        s2[:, j * 512:(j + 1) * 512],
                        lhsT=qkT["kT22"][psl, sbj * 128:(sbj + 1) * 128],
                        rhs=qkT["qT22"][psl, tsl],
                        start=True,
                        stop=True,
                    )
                sbm0 = sb0 * 128 - tci * TCW
                if sbm0 >= 256:
                    for j in range(2):
                        lo = sbm0 + j * 128
                        nc.scalar.activation(
                            ep[:, 2, j, lo:TCW],
                            s2[:, j * 512 + lo:(j + 1) * 512],
                            EXPF,
                            bias=expb[:, 0:1],
                        )
                else:
                    nc.scalar.activation(
                        ep[:, 2, :, :],
                        s2.rearrange("p (j t) -> p j t", j=2),
                        EXPF,
                        bias=expb[:, 0:1],
                    )
                for j in range(2):
                    sbmj = (sb0 + j) * 128 - tci * TCW
                    if sbmj >= 0:
                        if sbmj > 0:
                            nc.vector.memset(ep[:, 2, j, 0:sbmj].bitcast(I32), 0)
                        nc.vector.tensor_mul(
                            ep[:, 2, j, sbmj:sbmj + 128],
                            ep[:, 2, j, sbmj:sbmj + 128],
                            ztt[:, 384:512],
                        )
                # --- PV: one DoubleRow matmul per head covers both sbs
                #     (bf16 path: two normal matmuls) ---
                for h in range(HPC):
                    if is_bf:
                        for j in range(2):
                            nc.tensor.matmul(
                                cps[h][0:66, :],
                                lhsT=v_bf[:, sb0 + j, h * 66:(h + 1) * 66],
                                rhs=ep[:, h, j, :],
                                start=(sb0 + j == 0),
                                stop=(sb0 + j == nsb - 1),
                            )
                    else:
                        nc.tensor.matmul(
                            cps[h][0:66, :],
                            lhsT=v_sb[:, sb0:sb0 + 2, h * 66:(h + 1) * 66],
                            rhs=ep[:, h, :, :],
                            start=(pr == 0),
                            stop=(pr == nsb // 2 - 1),
                            perf_mode=DR,
                        )
            # --- normalize: evac unnormalized ctx+sums; fast reciprocal;
            #     broadcast 1/sums to 64 partitions via a one-hot PE matmul ---
            cues = []
            if tci == NTC - 1:
                # keep the PE HAM-warm through the normalize chain so the
                # final out-projection runs at 2.4 GHz
                dum_ps = ps_s.tile([128, 1024], F32, tag="S", name="dumps")
                for dk in range(8):
                    nc.tensor.matmul(
                        dum_ps[:, 0:512],
                        lhsT=zt_sb[:, 0:128],
                        rhs=zt_sb[:, 0:512],
                        start=(dk == 0),
                        stop=(dk == 7),
                    )
            sall = rpool.tile([HPC, TCW], F32, tag="sall", name=f"sall{tci}")
            for h in range(HPC):
                cue = rpool.tile([66, TCW], F32, tag=f"cue{h}", name=f"cue{tci}_{h}")
                if h == 1 and tci == NTC - 1:  # scalar is idle on the tail
                    nc.scalar.activation(cue[:, :], cps[h][0:66, :], COPYF)
                else:
                    nc.vector.tensor_copy(out=cue[:, :], in_=cps[h][0:66, :])
                nc.sync.dma_start(out=sall[h:h + 1, :], in_=cue[64:65, :])
                cues.append(cue)
            rall = rpool.tile([HPC, TCW], F32, tag="rall", name=f"rall{tci}")
            nc.vector.reciprocal_approx_fast(out=rall[:, :], in_=sall[:, :])
            rall_bf = rpool.tile([HPC, TCW], BF16, tag="rallbf", name=f"rbf{tci}")
            nc.vector.tensor_copy(out=rall_bf[:, :], in_=rall[:, :])
            # broadcast h0 -> rows 0:64 and h1 -> rows 64:128 of one bank
            # (concurrent col-tiled matmuls), then h2 on the next rotation.
            rec01 = ps_o.tile([128, TCW], F32, tag="O", name=f"rec01_{tci}")
            for hh in range(2):
                nc.tensor.matmul(
                    rec01[hh * 64:(hh + 1) * 64, :],
                    lhsT=sel_sb[:, hh * 64:(hh + 1) * 64],
                    rhs=rall_bf[:, :],
                    start=True,
                    stop=True,
                )
            nc.vector.tensor_mul(ctxT[0][:, tsl], cues[0][0:64, :], rec01[0:64, :])
            nc.vector.tensor_mul(ctxT[1][:, tsl], cues[1][0:64, :], rec01[64:128, :])
            psx = ps_c if tci == NTC - 1 else ps_o
            tgx = "ctx" if tci == NTC - 1 else "O"
            rec2 = psx.tile([128, TCW], F32, tag=tgx, name=f"rec2_{tci}")
            nc.tensor.matmul(
                rec2[0:64, :],
                lhsT=sel_sb[:, 128:192],
                rhs=rall_bf[:, :],
                start=True,
                stop=True,
            )
            nc.vector.tensor_mul(ctxT[2][:, tsl], cues[2][0:64, :], rec2[0:64, :])
            # --- output projection for this t-chunk; single DMA out ---
            osb = opool.tile([128, 4 * 384], F32, tag="osb", name=f"osb{tci}")
            for j, tb in enumerate(range(4 * tci, 4 * tci + 4)):
                po = psx.tile([128, 512], F32, tag=tgx, name=f"po{tb}")
                for h in range(HPC):
                    nc.tensor.matmul(
                        po[:, 0:384],
                        lhsT=ctxT[h][:, tb * 128:(tb + 1) * 128],
                        rhs=wo_sb[h][:, :],
                        start=(h == 0),
                        stop=(h == HPC - 1),
                    )
                dst = osb[:, j * 384:(j + 1) * 384]
                if j % 2 == 0:
                    nc.vector.tensor_copy(out=dst, in_=po[:, 0:384])
                else:
                    nc.scalar.activation(dst, po[:, 0:384], COPYF)
                if tci == NTC - 1:
                    nc.sync.dma_start(
                        out=out[tb * 128:(tb + 1) * 128, :],
                        in_=osb[:, j * 384:(j + 1) * 384],
                    )
                elif j == 1 or j == 3:
                    jh = j - 1
                    nc.sync.dma_start(
                        out=out[
                            tci * 512 + jh * 128:tci * 512 + (jh + 2) * 128, :
                        ].rearrange("(jj p) c -> p jj c", p=128),
                        in_=osb.rearrange("p (jj c) -> p jj c", jj=4)[
                            :, jh:jh + 2, :
                        ],
                    )

    return nc


def get_nc():
    global _CACHED_NC
    if _CACHED_NC is None:
        nc = build_nc()
        nc.finalize()
        _CACHED_NC = nc
    return _CACHED_NC


def make_core_inputs(x, Wq, bq, Wk, bk, Wv, bv, Wo, bo):
    """Host-side shard prep. Returns (in_maps, host_add) where host_add[384]
    is added to every output row (exact fold of bv/bo)."""
    scale = 1.0 / math.sqrt(D)
    assert np.all(bq == 0.0) and np.all(bk == 0.0), "kernel assumes bq=bk=0"
    host_add = (bv.astype(np.float64) @ Wo.astype(np.float64) + bo).astype(np.float32)

    si = np.arange(128)[:, None]
    tj = np.arange(128)[None, :]
    zt = np.zeros((128, 512), dtype=np.float32)
    zt[:, 384:512] = (si <= tj).astype(np.float32)

    sel = np.zeros((HPC, HPC * D), dtype=np.float32)
    for h in range(HPC):
        sel[h, h * D:(h + 1) * D] = 1.0

    in_maps = []
    for core in range(NCORES):
        b = core // 2
        h0 = HPC * (core % 2)  # first head (0 or 3)
        cs = slice(h0 * D, (h0 + HPC) * D)
        wq_s = (Wq[:, cs] * scale).astype(np.float32)
        wk_s = Wk[:, cs].astype(np.float32)
        wqk = np.concatenate(
            [
                wq_s[:, 0:128],
                wk_s[:, 0:128],
                np.tile(wq_s[:, 128:192], (1, 2)),
                np.tile(wk_s[:, 128:192], (1, 2)),
            ],
            axis=1,
        )
        in_maps.append(
            {
                "xt": np.ascontiguousarray(x[b].T).astype(BF16NP),
                "wqk": np.ascontiguousarray(wqk).astype(BF16NP),
                "wv": np.ascontiguousarray(Wv[:, cs]).astype(BF16NP),
                "wo": np.ascontiguousarray(Wo[cs, :]).astype(BF16NP),
                "zt": zt.astype(BF16NP),
                "sel": sel.astype(BF16NP),
            }
        )
    return in_maps, host_add


def kernel(x, Wq, bq, Wk, bk, Wv, bv, Wo, bo, _trace=False):
    x = np.asarray(x, dtype=np.float32)
    Wq, bq = np.asarray(Wq, np.float32), np.asarray(bq, np.float32)
    Wk, bk = np.asarray(Wk, np.float32), np.asarray(bk, np.float32)
    Wv, bv = np.asarray(Wv, np.float32), np.asarray(bv, np.float32)
    Wo, bo = np.asarray(Wo, np.float32), np.asarray(bo, np.float32)

    from concourse.bass_utils import run_bass_kernel_spmd

    nc = get_nc()
    in_maps, host_add = make_core_inputs(x, Wq, bq, Wk, bk, Wv, bv, Wo, bo)
    res = run_bass_kernel_spmd(
        nc, in_maps, core_ids=list(range(NCORES)), trace=_trace
    )
    out = np.empty((B, T, C), dtype=np.float32)
    for b in range(B):
        out[b] = res.results[2 * b]["out"] + res.results[2 * b + 1]["out"] + host_add
    if _trace:
        return out, res
    return out


# revision 18
# speedup vs baseline: 1.2491x; 1.0042x over previous
"""Causal multi-head attention (B=4, T=2048, C=384, 6 heads of 64) on 8 trn2 cores.

Sharding: 24 (batch, head) pairs -> 8 cores; core c handles batch c//2 and
heads [3*(c%2), 3*(c%2)+3). Each core computes q/k/v projections for its 3
heads, causal softmax(q k^T / 8) v, and a PARTIAL output projection
ctx_heads @ Wo_heads. Host sums the two partials per batch and adds the
exactly-folded bias terms (bv @ Wo + bo; softmax weights sum to 1 so a v-bias
contributes bv @ Wo to every row).

Requires bq == bk == 0 (true for this problem: spec fill=zeros).

v2 structural changes vs baseline:
 - softmax normalization: sums gathered via direct SBUF->SBUF DMA,
   reciprocal_approx_fast (5x faster than iterative divide), and the
   64-partition broadcast done with a tiny PE matmul (one-hot selector)
   instead of DRAM round-trip DMAs.
 - PSUM->SBUF evacuations split between ScalarE and VectorE.
 - input DMAs batched (3 for x^T, 1 per weight tensor); vones -> memset.
 - one output DMA per t-chunk (4 total) instead of 16.
 - dummy exp at start so the ACT table load happens during the DMA phase.
"""

import math
from contextlib import ExitStack

import ml_dtypes
import numpy as np

BF16NP = ml_dtypes.bfloat16

B, T, C = 4, 2048, 384
NH, D = 6, 64          # total heads, head dim
HPC = 3                # heads per core
NCORES = 8
NKC = C // 128         # 3 contraction chunks for the projections
NTB = T // 128         # 16 row blocks
TCW = 512              # t-chunk width for the attention loop
NTC = T // TCW         # 4 t-chunks

_CACHED_NC = None


def build_nc():
    import concourse.bass as bass
    import concourse.mybir as mybir
    import concourse.tile as tile
    from concourse import bacc

    F32 = mybir.dt.float32
    F32R = mybir.dt.float32r
    FP8 = mybir.dt.float8e4
    I32 = mybir.dt.int32
    DR = mybir.MatmulPerfMode.DoubleRow
    EXPB = -1.5  # global shift: keeps exp(s) < 448 for fp8; cancels in softmax
    BF16 = mybir.dt.bfloat16
    EXPF = mybir.ActivationFunctionType.Exp
    COPYF = mybir.ActivationFunctionType.Copy

    nc = bacc.Bacc("TRN2", target_bir_lowering=False, debug=False)

    xt = nc.dram_tensor("xt", [C, T], BF16, kind="ExternalInput")
    wqk = nc.dram_tensor("wqk", [C, 512], BF16, kind="ExternalInput")
    wv = nc.dram_tensor("wv", [C, 192], BF16, kind="ExternalInput")
    wo = nc.dram_tensor("wo", [HPC * D, 384], BF16, kind="ExternalInput")
    zt = nc.dram_tensor("zt", [128, 512], BF16, kind="ExternalInput")
    sel = nc.dram_tensor("sel", [HPC, HPC * D], BF16, kind="ExternalInput")
    out = nc.dram_tensor("out", [T, C], F32, kind="ExternalOutput")

    with ExitStack() as ctx:
        tc = ctx.enter_context(tile.TileContext(nc))
        const = ctx.enter_context(tc.tile_pool(name="const", bufs=1))
        xpool = ctx.enter_context(tc.tile_pool(name="xp", bufs=1))
        qkpool = ctx.enter_context(tc.tile_pool(name="qkp", bufs=1))
        vpool = ctx.enter_context(tc.tile_pool(name="vp", bufs=1))
        expp = ctx.enter_context(tc.tile_pool(name="expp", bufs=6))
        cxp = ctx.enter_context(tc.tile_pool(name="cxp", bufs=1))
        rpool = ctx.enter_context(tc.tile_pool(name="rp", bufs=2))
        opool = ctx.enter_context(tc.tile_pool(name="op", bufs=2))
        ps_s = ctx.enter_context(tc.tile_pool(name="ps_s", bufs=2, space="PSUM"))
        ps_c = ctx.enter_context(tc.tile_pool(name="ps_c", bufs=3, space="PSUM"))
        ps_o = ctx.enter_context(tc.tile_pool(name="ps_o", bufs=1, space="PSUM"))

        # ---- constants (per-kc DMAs: finer deps for an early first matmul) ----
        wqk_all = const.tile([128, NKC * 512], BF16, tag="wqk")
        for kc in range(NKC):
            nc.sync.dma_start(
                out=wqk_all[:, kc * 512:(kc + 1) * 512],
                in_=wqk[kc * 128:(kc + 1) * 128, :],
            )
        wqk_sb = [wqk_all[:, kc * 512:(kc + 1) * 512] for kc in range(NKC)]

        zt_sb = const.tile([128, 512], BF16, tag="zt")
        nc.sync.dma_start(out=zt_sb, in_=zt[:, :])
        zt8 = const.tile([128, 512], FP8, tag="zt8")
        nc.vector.tensor_copy(out=zt8[:, :], in_=zt_sb[:, :])
        sel_sb = const.tile([HPC, HPC * D], BF16, tag="sel")
        nc.sync.dma_start(out=sel_sb, in_=sel[:, :])

        expb = const.tile([128, 1], F32, tag="expb")
        nc.vector.memset(expb[:, :], EXPB)
        onesr = const.tile([128, 64], BF16, tag="onesr")
        nc.vector.memset(onesr[:, :], 1.0)

        # dummy exp: forces the ACT table load during the DMA phase
        dumm = const.tile([1, 8], BF16, tag="dumm")
        nc.scalar.activation(dumm[:, :], zt_sb[0:1, 0:8], EXPF)

        # ---- x^T in SBUF: 3 [128, 2048] row chunks (2 DMAs each) ----
        xts = []
        for kc in range(NKC):
            t_ = xpool.tile([128, T], BF16, tag=f"xt{kc}")
            nc.sync.dma_start(out=t_[:, 0:1024], in_=xt[kc * 128:(kc + 1) * 128, 0:1024])
            nc.sync.dma_start(out=t_[:, 1024:T], in_=xt[kc * 128:(kc + 1) * 128, 1024:T])
            xts.append(t_)

        wv_all = const.tile([128, NKC * 192], BF16, tag="wv")
        nc.sync.dma_start(
            out=wv_all.rearrange("p (kc c) -> p kc c", kc=NKC),
            in_=wv.rearrange("(kc p) c -> p kc c", p=128),
        )
        wv_sb = [wv_all[:, kc * 192:(kc + 1) * 192] for kc in range(NKC)]

        wo_all = const.tile([64, HPC * 384], BF16, tag="wo")
        nc.sync.dma_start(
            out=wo_all.rearrange("p (h c) -> p h c", h=HPC),
            in_=wo.rearrange("(h p) c -> p h c", p=64),
        )
        wo_sb = [wo_all[:, h * 384:(h + 1) * 384] for h in range(HPC)]

        # ---- projections: qT/kT packed [d(2 heads), T] ----
        # wqk columns: mt0=[q0|q1] mt1=[k0|k1] mt2=[q2|q2] mt3=[k2|k2]
        # (q columns pre-scaled by 1/sqrt(D) on host)
        qk_names = ["qT01", "kT01", "qT22", "kT22"]
        qkT = {}
        for mt, name in enumerate(qk_names):
            qkT[name] = qkpool.tile([128, T], BF16, tag=name, name=name)
        for mt, name in enumerate(qk_names):
            for nch in range(4):
                ps = ps_s.tile([128, 1024], F32, tag="S")
                for kc in range(NKC):
                    nc.tensor.matmul(
                        ps[:, 0:512],
                        lhsT=wqk_sb[kc][:, mt * 128:(mt + 1) * 128],
                        rhs=xts[kc][:, nch * 512:(nch + 1) * 512],
                        start=(kc == 0),
                        stop=(kc == NKC - 1),
                    )
                dst = qkT[name][:, nch * 512:(nch + 1) * 512]
                if (mt * 4 + nch) % 2 == 0:
                    nc.vector.tensor_copy(out=dst, in_=ps[:, 0:512])
                else:
                    nc.scalar.activation(dst, ps[:, 0:512], COPYF)

        # ---- v in natural [s, d'] layout + ones column per head (66 wide) ----
        v_sb = vpool.tile([128, NTB, 208], FP8, tag="vsb")
        ones_view = v_sb[:, :, 0:198].rearrange("p s (h e) -> p s h e", e=66)[:, :, :, 64:66]
        nc.vector.memset(ones_view, 1.0)
        v_bf = vpool.tile([128, 4, 3 * 66], BF16, tag="vbf")
        ones_bf = v_bf.rearrange("p s (h e) -> p s h e", e=66)[:, :, :, 64:66]
        nc.vector.memset(ones_bf, 1.0)
        for tb in range(NTB):
            ps = ps_s.tile([128, 1024], F32, tag="S")
            for kc in range(NKC):
                nc.tensor.matmul(
                    ps[:, 0:192],
                    lhsT=xts[kc][:, tb * 128:(tb + 1) * 128],
                    rhs=wv_sb[kc][:, :],
                    start=(kc == 0),
                    stop=(kc == NKC - 1),
                )
            dst = v_sb[:, tb, 0:198].rearrange("p (h e) -> p h e", e=66)[:, :, 0:64]
            src = ps[:, 0:192].rearrange("p (h e) -> p h e", e=64)
            if tb % 2 == 0:
                nc.vector.tensor_copy(out=dst, in_=src)
            else:
                nc.scalar.activation(dst, src, COPYF)
            if tb < 4:
                dbf = v_bf[:, tb, :].rearrange("p (h e) -> p h e", e=66)[:, :, 0:64]
                if tb % 2 == 0:
                    nc.scalar.activation(dbf, src, COPYF)
                else:
                    nc.vector.tensor_copy(out=dbf, in_=src)

        # ---- per-head normalized ctx^T [64, T] ----
        ctxT = [
            cxp.tile([64, T], BF16, tag=f"ctxT{h}", name=f"ctxT{h}")
            for h in range(HPC)
        ]

        def mask_exp_block(e_tile, col0, sbm):
            """Causal-mask the exp'd S^T block at e_tile[:, col0:col0+512].

            sbm = s_block_start - t_chunk_start (>= 0 on diagonal blocks).
            Cols [col0, col0+sbm) are fully above the diagonal (-> *0) and
            cols [col0+sbm, col0+sbm+128) are triangular; zt_sb is laid out
            as [384 zeros | 128-wide triangle] so one suffix slice covers
            both regions in a single multiply."""
            w = sbm + 128
            nc.vector.tensor_mul(
                e_tile[:, col0:col0 + w],
                e_tile[:, col0:col0 + w],
                zt_sb[:, 512 - w:512],
            )

        for tci in range(NTC):
            tsl = slice(tci * TCW, (tci + 1) * TCW)
            nsb = 4 * tci + 4
            cps = [
                ps_c.tile([128, TCW], F32, tag="ctx", name=f"cps{tci}_{h}")
                for h in range(HPC)
            ]
            is_bf = tci == 0  # first t-chunk: few attended positions; fp8
            # V-noise does not average out -> keep it in bf16
            for pr in range(nsb // 2):
                sb0 = 2 * pr
                # epair layout: [p, head, j(sb parity), t]
                if is_bf:
                    ep = expp.tile([128, HPC, 2, TCW], BF16, tag="E0", bufs=2)
                    ztt = zt_sb
                else:
                    ep = expp.tile([128, HPC, 2, TCW], FP8, tag="E")
                    ztt = zt8
                # --- heads 0,1: per sb, row-tiled score pair (K=64 each) ---
                for j in range(2):
                    sbj = sb0 + j
                    sbm = sbj * 128 - tci * TCW  # >= 0 on diagonal blocks
                    s01 = ps_s.tile([128, 1024], F32, tag="S")
                    lo = max(sbm, 0)  # cols [0, lo) are fully masked
                    for hh in range(2):
                        psl = slice(hh * 64, (hh + 1) * 64)
                        nc.tensor.matmul(
                            s01[:, hh * 512 + lo:(hh + 1) * 512],
                            lhsT=qkT["kT01"][psl, sbj * 128:(sbj + 1) * 128],
                            rhs=qkT["qT01"][psl, tci * TCW + lo:(tci + 1) * TCW],
                            start=True,
                            stop=True,
                        )
                    nc.scalar.activation(
                        ep[:, 0:2, j, lo:TCW],
                        s01.rearrange("p (h t) -> p h t", h=2)[:, :, lo:TCW],
                        EXPF,
                        bias=expb[:, 0:1],
                    )
                    if sbm >= 0:
                        for hh in range(2):
                            if sbm > 0:
                                nc.vector.memset(
                                    ep[:, hh, j, 0:sbm].bitcast(I32), 0
                                )
                            nc.vector.tensor_mul(
                                ep[:, hh, j, sbm:sbm + 128],
                                ep[:, hh, j, sbm:sbm + 128],
                                ztt[:, 384:512],
                            )
                # --- head 2: both sbs of the pair, row-tiled (dup packing) ---
                s2 = ps_s.tile([128, 1024], F32, tag="S")
                for j in range(2):
                    sbj = sb0 + j
                    loj = max(sbj * 128 - tci * TCW, 0)
                    psl = slice(j * 64, (j + 1) * 64)
                    nc.tensor.matmul(
                        s2[:, j * 512 + loj:(j + 1) * 512],
                        lhsT=qkT["kT22"][psl, sbj * 128:(sbj + 1) * 128],
                        rhs=qkT["qT22"][psl, tci * TCW + loj:(tci + 1) * TCW],
                        start=True,
                        stop=True,
                    )
                sbm0 = sb0 * 128 - tci * TCW
                if sbm0 >= 0:
                    for j in range(2):
                        lo = sbm0 + j * 128
                        nc.scalar.activation(
                            ep[:, 2, j, lo:TCW],
                            s2[:, j * 512 + lo:(j + 1) * 512],
                            EXPF,
                            bias=expb[:, 0:1],
                        )
                else:
                    nc.scalar.activation(
                        ep[:, 2, :, :],
                        s2.rearrange("p (j t) -> p j t", j=2),
                        EXPF,
                        bias=expb[:, 0:1],
                    )
                for j in range(2):
                    sbmj = (sb0 + j) * 128 - tci * TCW
                    if sbmj >= 0:
                        if sbmj > 0:
                            nc.vector.memset(ep[:, 2, j, 0:sbmj].bitcast(I32), 0)
                        nc.vector.tensor_mul(
                            ep[:, 2, j, sbmj:sbmj + 128],
                            ep[:, 2, j, sbmj:sbmj + 128],
                            ztt[:, 384:512],
                        )
                # --- PV: one DoubleRow matmul per head covers both sbs
                #     (bf16 path: two normal matmuls) ---
                lo0 = max(sb0 * 128 - tci * TCW, 0)
                for h in range(HPC):
                    if is_bf:
                        for j in range(2):
                            loj = max((sb0 + j) * 128 - tci * TCW, 0)
                            nc.tensor.matmul(
                                cps[h][0:66, loj:TCW],
                                lhsT=v_bf[:, sb0 + j, h * 66:(h + 1) * 66],
                                rhs=ep[:, h, j, loj:TCW],
                                start=(sb0 + j == 0),
                                stop=(sb0 + j == nsb - 1),
                            )
                    else:
                        nc.tensor.matmul(
                            cps[h][0:66, lo0:TCW],
                            lhsT=v_sb[:, sb0:sb0 + 2, h * 66:(h + 1) * 66],
                            rhs=ep[:, h, :, lo0:TCW],
                            start=(pr == 0),
                            stop=(pr == nsb // 2 - 1),
                            perf_mode=DR,
                        )
            # --- normalize: evac unnormalized ctx+sums; fast reciprocal;
            #     broadcast 1/sums to 64 partitions via a one-hot PE matmul ---
            cues = []
            if tci == NTC - 1:
                # keep the PE HAM-warm through the normalize chain so the
                # final out-projection runs at 2.4 GHz
                dum_ps = ps_s.tile([128, 1024], F32, tag="S", name="dumps")
                for dk in range(8):
                    nc.tensor.matmul(
                        dum_ps[:, 0:512],
                        lhsT=zt_sb[:, 0:128],
                        rhs=zt_sb[:, 0:512],
                        start=(dk == 0),
                        stop=(dk == 7),
                    )
            sall = rpool.tile([HPC, TCW], F32, tag="sall", name=f"sall{tci}")
            for h in range(HPC):
                cue = rpool.tile([66, TCW], F32, tag=f"cue{h}", name=f"cue{tci}_{h}")
                if h == 1 and tci == NTC - 1:  # scalar is idle on the tail
                    nc.scalar.activation(cue[:, :], cps[h][0:66, :], COPYF)
                else:
                    nc.vector.tensor_copy(out=cue[:, :], in_=cps[h][0:66, :])
                nc.sync.dma_start(out=sall[h:h + 1, :], in_=cue[64:65, :])
                cues.append(cue)
            rall = rpool.tile([HPC, TCW], F32, tag="rall", name=f"rall{tci}")
            nc.vector.reciprocal_approx_fast(out=rall[:, :], in_=sall[:, :])
            rall_bf = rpool.tile([HPC, TCW], BF16, tag="rallbf", name=f"rbf{tci}")
            nc.vector.tensor_copy(out=rall_bf[:, :], in_=rall[:, :])
            rec01 = ps_o.tile([128, TCW], F32, tag="O", name=f"rec01_{tci}")
            for hh in range(2):
                nc.tensor.matmul(
                    rec01[hh * 64:(hh + 1) * 64, :],
                    lhsT=sel_sb[:, hh * 64:(hh + 1) * 64],
                    rhs=rall_bf[:, :],
                    start=True,
                    stop=True,
                )
            nc.vector.tensor_mul(ctxT[0][:, tsl], cues[0][0:64, :], rec01[0:64, :])
            nc.vector.tensor_mul(ctxT[1][:, tsl], cues[1][0:64, :], rec01[64:128, :])
            psx = ps_c if tci == NTC - 1 else ps_o
            tgx = "ctx" if tci == NTC - 1 else "O"
            rec2 = psx.tile([128, TCW], F32, tag=tgx, name=f"rec2_{tci}")
            nc.tensor.matmul(
                rec2[0:64, :],
                lhsT=sel_sb[:, 128:192],
                rhs=rall_bf[:, :],
                start=True,
                stop=True,
            )
            nc.vector.tensor_mul(ctxT[2][:, tsl], cues[2][0:64, :], rec2[0:64, :])
            # --- output projection for this t-chunk; single DMA out ---
            osb = opool.tile([128, 4 * 384], F32, tag="osb", name=f"osb{tci}")
            for j, tb in enumerate(range(4 * tci, 4 * tci + 4)):
                po = psx.tile([128, 512], F32, tag=tgx, name=f"po{tb}")
                for h in range(HPC):
                    nc.tensor.matmul(
                        po[:, 0:384],
                        lhsT=ctxT[h][:, tb * 128:(tb + 1) * 128],
                        rhs=wo_sb[h][:, :],
                        start=(h == 0),
                        stop=(h == HPC - 1),
                    )
                dst = osb[:, j * 384:(j + 1) * 384]
                if tci == NTC - 1 and j % 2 == 1:
                    nc.scalar.activation(dst, po[:, 0:384], COPYF)
                else:
                    nc.vector.tensor_copy(out=dst, in_=po[:, 0:384])
                if tci == NTC - 1:
                    nc.sync.dma_start(
                        out=out[tb * 128:(tb + 1) * 128, :],
                        in_=osb[:, j * 384:(j + 1) * 384],
                    )
                elif j == 1 or j == 3:
                    jh = j - 1
                    nc.sync.dma_start(
                        out=out[
                            tci * 512 + jh * 128:tci * 512 + (jh + 2) * 128, :
                        ].rearrange("(jj p) c -> p jj c", p=128),
                        in_=osb.rearrange("p (jj c) -> p jj c", jj=4)[
                            :, jh:jh + 2, :
                        ],
                    )

    return nc


def get_nc():
    global _CACHED_NC
    if _CACHED_NC is None:
        nc = build_nc()
        nc.finalize()
        _CACHED_NC = nc
    return _CACHED_NC


def make_core_inputs(x, Wq, bq, Wk, bk, Wv, bv, Wo, bo):
    """Host-side shard prep. Returns (in_maps, host_add) where host_add[384]
    is added to every output row (exact fold of bv/bo)."""
    scale = 1.0 / math.sqrt(D)
    assert np.all(bq == 0.0) and np.all(bk == 0.0), "kernel assumes bq=bk=0"
    host_add = (bv.astype(np.float64) @ Wo.astype(np.float64) + bo).astype(np.float32)

    si = np.arange(128)[:, None]
    tj = np.arange(128)[None, :]
    zt = np.zeros((128, 512), dtype=np.float32)
    zt[:, 384:512] = (si <= tj).astype(np.float32)

    sel = np.zeros((HPC, HPC * D), dtype=np.float32)
    for h in range(HPC):
        sel[h, h * D:(h + 1) * D] = 1.0

    in_maps = []
    for core in range(NCORES):
        b = core // 2
        h0 = HPC * (core % 2)  # first head (0 or 3)
        cs = slice(h0 * D, (h0 + HPC) * D)
        wq_s = (Wq[:, cs] * scale).astype(np.float32)
        wk_s = Wk[:, cs].astype(np.float32)
        wqk = np.concatenate(
            [
                wq_s[:, 0:128],
                wk_s[:, 0:128],
                np.tile(wq_s[:, 128:192], (1, 2)),
                np.tile(wk_s[:, 128:192], (1, 2)),
            ],
            axis=1,
        )
        in_maps.append(
            {
                "xt": np.ascontiguousarray(x[b].T).astype(BF16NP),
                "wqk": np.ascontiguousarray(wqk).astype(BF16NP),
                "wv": np.ascontiguousarray(Wv[:, cs]).astype(BF16NP),
                "wo": np.ascontiguousarray(Wo[cs, :]).astype(BF16NP),
                "zt": zt.astype(BF16NP),
                "sel": sel.astype(BF16NP),
            }
        )
    return in_maps, host_add


def kernel(x, Wq, bq, Wk, bk, Wv, bv, Wo, bo, _trace=False):
    x = np.asarray(x, dtype=np.float32)
    Wq, bq = np.asarray(Wq, np.float32), np.asarray(bq, np.float32)
    Wk, bk = np.asarray(Wk, np.float32), np.asarray(bk, np.float32)
    Wv, bv = np.asarray(Wv, np.float32), np.asarray(bv, np.float32)
    Wo, bo = np.asarray(Wo, np.float32), np.asarray(bo, np.float32)

    from concourse.bass_utils import run_bass_kernel_spmd

    nc = get_nc()
    in_maps, host_add = make_core_inputs(x, Wq, bq, Wk, bk, Wv, bv, Wo, bo)
    res = run_bass_kernel_spmd(
        nc, in_maps, core_ids=list(range(NCORES)), trace=_trace
    )
    out = np.empty((B, T, C), dtype=np.float32)
    for b in range(B):
        out[b] = res.results[2 * b]["out"] + res.results[2 * b + 1]["out"] + host_add
    if _trace:
        return out, res
    return out
